# revision 27
# baseline (speedup 1.0000x reference)
"""MoE layer (E=8 experts, top-2, D=1024, H=4096, N=4096 tokens) on 8 TRN2
NeuronCores.

Strategy: expert-parallel with load balancing. The router (gate matmul +
softmax + top-2) is ~0.1% of the FLOPs and runs on host in float64
(verified to reproduce the reference's f32 top-2 selection exactly). The
host gathers each expert's tokens and ships them to the cores; each core
runs dense single-expert FFN passes over fixed-size token slots:

    outT = w2.T @ gelu(w1.T @ xT + b1)

Only the top-2 experts per token are computed (vs all 8 in the dense
data-parallel formulation) — a ~3.6x FLOP cut. The host applies the
combine weights and the (comb-weighted) b2 bias during the scatter-add
back to token order.

Load balancing: expert loads vary (~944..1091 here), so one-expert-per-
core would pad every core to the max. Instead experts are sorted by load
and paired big-with-small; each pair is split across two cores, so every
core processes slot1 = half of a heavy expert (s1 tokens) + slot2 = half
of a light expert (s2 tokens), with uniform (s1, s2) across cores (SPMD:
one NEFF). Capacity s1+s2 ~= ceil(max_heavy/2) + ceil(max_light/2) beats
max_e count_e.

Layout: activations keep the feature dim on partitions (tokens on the
free axis) so both matmuls consume pre-tiled weights with no on-device
transposes:
    hT[h, t]   = sum_d w1[d, h] * xT[d, t]     (lhsT = w1 tile, rhs = xT)
    outT[d, t] = sum_h w2[h, d] * geluT[h, t]  (lhsT = w2 tile, rhs = geluT)

Matmuls run in bf16 (1 PE row/cycle, FWL weight loads) with fp32 PSUM
accumulation; measured end-to-end error vs the f32 reference is ~3.4e-3
absmax-relative, well under the 2e-2 gate.

Perf notes (measured on TRN2):
 - PE back-to-back MM issue gap = N/2.4GHz + ~2.5ns; LDWEIGHTS fully
   hidden by the PE reorder window. Keep SBUF row pitches and chunk
   starts 16B-aligned or streaming slows ~1.2x.
 - DMA instructions issue serially (~0.65us each) per issuing queue
   (sync/scalar); transfers stripe over 16 HW queues at ~190 GB/s
   aggregate. Emission order is chosen so nothing the PE needs early
   contends with bulk weight traffic.
 - Engines start ~7.2us into the NEFF; a short train of dependency-free
   warmup matmuls keeps the PE clock (HAM) at 2.4 GHz from the start.
"""

import numpy as np
import ml_dtypes

import concourse.bass as bass  # noqa: F401  (bass types used via tile/bacc)
import concourse.mybir as mybir
import concourse.tile as tile
from concourse import bacc, bass_utils

F32 = mybir.dt.float32
BF16 = mybir.dt.bfloat16
AFT = mybir.ActivationFunctionType
NPBF16 = np.dtype(ml_dtypes.bfloat16)

E = 8          # experts
D = 1024       # model dim
H = 4096       # expert hidden dim
P = 128        # partitions
NCORES = 8
NTOK = 4096    # total tokens (B*T = 2*2048)
KD = D // P    # 8 contraction chunks of D
NH = H // P    # 32 h tiles
ND = D // P    # 8 d tiles
NSLOT = 2      # expert slots per core (big-half + small-half)

_NC = {}       # compiled modules keyed by (s1, s2)


def _chunks(s):
    """Split a slot of s tokens into psum-sized chunks (<=512 f32/bank),
    8-aligned starts so SBUF addresses stay 16B-aligned in bf16."""
    if s <= 512:
        return [s]
    a = -(-(s // 2) // 8) * 8
    return [a, s - a]


def _build(s1, s2):
    c = s1 + s2
    slot_off = [0, s1]
    slot_chunks = [_chunks(s1), _chunks(s2)]
    nc = bacc.Bacc("TRN2", target_bir_lowering=False, debug=False,
                   num_devices=NCORES)
    xT = nc.dram_tensor("xT", [P, KD, c], BF16, kind="ExternalInput").ap()
    w1t = nc.dram_tensor("w1t", [NSLOT, NH, P, KD, P], BF16,
                         kind="ExternalInput").ap()
    b1t = nc.dram_tensor("b1t", [P, NSLOT, NH], F32,
                         kind="ExternalInput").ap()
    w2t = nc.dram_tensor("w2t", [NSLOT, ND, P, NH, P], BF16,
                         kind="ExternalInput").ap()
    outT = nc.dram_tensor("outT", [P, ND, c], F32, kind="ExternalOutput").ap()

    with tile.TileContext(nc) as tc:
        with (
            tc.tile_pool(name="const", bufs=1) as cpool,
            tc.tile_pool(name="w1p", bufs=NH) as w1p,
            tc.tile_pool(name="w2p", bufs=2) as w2p,
            tc.tile_pool(name="otp", bufs=4) as otp,
            tc.tile_pool(name="ps", bufs=7, space="PSUM") as ps,
            tc.tile_pool(name="pw", bufs=1, space="PSUM") as pw,
        ):
            # ---- PE warmup: dependency-free matmuls from ~0.5us after
            # engine start keep the HAM activity window busy during the
            # input fill, so the real stream starts at 2.4 GHz.
            wsrc = cpool.tile([P, 256], BF16)
            nc.vector.memset(wsrc[:], 0.0)
            pwt = pw.tile([P, P], F32)
            for _ in range(38):
                nc.tensor.matmul(pwt[:], wsrc[:, 0:P], wsrc[:, P:256],
                                 start=True, stop=True)

            # ---- input DMAs. scalar queue: b1 + slot-1 xT (needed first);
            # sync queue: slot-1 w1 tiles, then slot-2 xT, then slot-2 w1
            # (its ring slots free up as slot-1 mm1 consumes tiles), then
            # (emitted in the loops below) w2 tiles and outputs.
            b1_s = cpool.tile([P, NSLOT, NH], F32)
            nc.scalar.dma_start(b1_s[:], b1t[:])
            xTt = cpool.tile([P, KD, c], BF16)
            for kd in range(KD):
                nc.scalar.dma_start(xTt[:, kd, 0:s1], xT[:, kd, 0:s1])
            w1_all = {}
            for s in range(NSLOT):
                if s == 1:
                    for kd in range(KD):
                        nc.sync.dma_start(xTt[:, kd, s1:c], xT[:, kd, s1:c])
                for h in range(NH):
                    w1_s = w1p.tile([P, KD, P], BF16)
                    nc.sync.dma_start(w1_s[:], w1t[s, h])
                    w1_all[s, h] = w1_s
            geluT = cpool.tile([P, NH, c], BF16)

            # ---- mm1 + gelu: geluT[h, t] = gelu(sum_d w1[d,h] x[d,t] + b1)
            for s in range(NSLOT):
                for h in range(NH):
                    if s == 0 and h < 8:
                        # the first h-groups are paced by the DMA fill; pad
                        # the PE's idle slivers with filler matmuls so the
                        # HAM never sees an idle window and re-throttles
                        for _ in range(3):
                            nc.tensor.matmul(pwt[:], wsrc[:, 0:P],
                                             wsrc[:, P:256],
                                             start=True, stop=True)
                    w1_s = w1_all.pop((s, h))
                    t0 = slot_off[s]
                    for cw in slot_chunks[s]:
                        ts = slice(t0, t0 + cw)
                        t0 += cw
                        ph = ps.tile([P, cw], F32, tag="ps")
                        for kd in range(KD):
                            nc.tensor.matmul(ph[:], w1_s[:, kd, :],
                                             xTt[:, kd, ts],
                                             start=(kd == 0),
                                             stop=(kd == KD - 1))
                        nc.scalar.activation(geluT[:, h, ts], ph[:], AFT.Gelu,
                                             bias=b1_s[:, s, h:h + 1])

            # ---- mm2: outT[d, t] = sum_h w2[h,d] geluT[h,t]
            for s in range(NSLOT):
                for d in range(ND):
                    w2_s = w2p.tile([P, NH, P], BF16)
                    for q in range(2):
                        nc.sync.dma_start(
                            w2_s[:, q * NH // 2:(q + 1) * NH // 2, :],
                            w2t[s, d, :, q * NH // 2:(q + 1) * NH // 2, :])
                    t0 = slot_off[s]
                    for cw in slot_chunks[s]:
                        ts = slice(t0, t0 + cw)
                        t0 += cw
                        po = ps.tile([P, cw], F32, tag="ps")
                        for hh in range(NH):
                            nc.tensor.matmul(po[:], w2_s[:, hh, :],
                                             geluT[:, hh, ts],
                                             start=(hh == 0),
                                             stop=(hh == NH - 1))
                        ot = otp.tile([P, cw], F32, tag="ot")
                        nc.vector.tensor_copy(ot[:], po[:])
                        nc.sync.dma_start(outT[:, d, ts], ot[:])

    nc.compile()
    return nc


def _get_nc(s1, s2):
    if (s1, s2) not in _NC:
        _NC[(s1, s2)] = _build(s1, s2)
    return _NC[(s1, s2)]


def _route(xf, gate_w, gate_b):
    """Top-2 routing in float64 (reproduces the reference's f32 decisions)."""
    lg = xf.astype(np.float64) @ gate_w.astype(np.float64) \
        + gate_b.astype(np.float64)
    lg -= lg.max(-1, keepdims=True)
    g = np.exp(lg)
    g /= g.sum(-1, keepdims=True)
    ti = np.argsort(-g, axis=-1, kind="stable")[:, :2]     # [N, 2] desc
    tg = np.take_along_axis(g, ti, axis=1)
    tg = tg / (tg.sum(-1, keepdims=True) + 1e-9)           # combine weights
    return ti, tg


def _wtiles(w1, b1, w2, e):
    w1te = np.ascontiguousarray(
        w1[e].reshape(KD, P, NH, P).transpose(2, 1, 0, 3)).astype(NPBF16)
    b1te = np.ascontiguousarray(b1[e].reshape(NH, P).T)
    w2te = np.ascontiguousarray(
        w2[e].reshape(NH, P, ND, P).transpose(2, 1, 0, 3)).astype(NPBF16)
    return w1te, b1te, w2te


def _prep(x, gate_w, gate_b, w1, b1, w2, b2):
    f = np.float32
    xf = np.asarray(x, f).reshape(NTOK, D)
    gate_w = np.asarray(gate_w, f)
    gate_b = np.asarray(gate_b, f)
    w1 = np.asarray(w1, f)
    b1 = np.asarray(b1, f)
    w2 = np.asarray(w2, f)
    b2 = np.asarray(b2, f)

    ti, tg = _route(xf, gate_w, gate_b)

    sels, wts = [], []
    for e in range(E):
        m = (ti == e)
        sel = np.nonzero(m.any(1))[0]                       # token ids, asc
        wt = tg[sel, m[sel].argmax(1)].astype(f)            # combine weight
        sels.append(sel)
        wts.append(wt)
    counts = np.array([len(s) for s in sels])

    # pair heavy experts with light ones; each pair spans two cores
    order = np.argsort(-counts, kind="stable")
    bigs, smalls = order[:E // 2], order[E // 2:]
    def _slot(maxc):                       # ceil(max/2), rounded up to 8
        return -(-((int(maxc) + 1) // 2) // 8) * 8
    s1 = _slot(counts[bigs].max())
    s2 = _slot(counts[smalls].max())
    c = s1 + s2

    wcache = {}
    in_maps, meta = [], []
    for p in range(E // 2):
        ea, eb = int(bigs[p]), int(smalls[p])
        for e in (ea, eb):
            if e not in wcache:
                wcache[e] = _wtiles(w1, b1, w2, e)
        halves = []
        for e in (ea, eb):
            n = len(sels[e])
            h1 = (n + 1) // 2
            halves.append([(sels[e][:h1], wts[e][:h1]),
                           (sels[e][h1:], wts[e][h1:])])
        for half in range(2):
            (sel_a, wt_a), (sel_b, wt_b) = halves[0][half], halves[1][half]
            xe = np.zeros((c, D), f)
            xe[0:len(sel_a)] = xf[sel_a]
            xe[s1:s1 + len(sel_b)] = xf[sel_b]
            xTe = np.ascontiguousarray(
                xe.T.reshape(KD, P, c).transpose(1, 0, 2)).astype(NPBF16)
            w1te = np.stack([wcache[ea][0], wcache[eb][0]])
            b1te = np.ascontiguousarray(
                np.stack([wcache[ea][1], wcache[eb][1]]).transpose(1, 0, 2))
            w2te = np.stack([wcache[ea][2], wcache[eb][2]])
            in_maps.append({"xT": xTe, "w1t": w1te, "b1t": b1te,
                            "w2t": w2te})
            meta.append(((ea, sel_a, wt_a, 0), (eb, sel_b, wt_b, s1)))
    return in_maps, meta, b2, s1, s2


def _assemble(results, meta, b2):
    out = np.zeros((NTOK, D), np.float32)
    for core, slots in enumerate(meta):
        yT = np.asarray(results[core]["outT"])              # [P, ND, c] f32
        y = yT.transpose(1, 0, 2).reshape(D, -1).T          # [c, D]
        for e, sel, wt, off in slots:
            if len(sel):
                out[sel] += wt[:, None] * (y[off:off + len(sel)] + b2[e])
    return out.reshape(2, NTOK // 2, D)


def run(inputs, trace=False):
    """Run the kernel; returns (output, exec_time_ns or None)."""
    in_maps, meta, b2, s1, s2 = _prep(**inputs)
    nc = _get_nc(s1, s2)
    res = bass_utils.run_bass_kernel_spmd(
        nc, in_maps, core_ids=list(range(NCORES)), trace=trace)
    return _assemble(res.results, meta, b2), res.exec_time_ns


def kernel(**inputs):
    out, _ = run(inputs, trace=False)
    return out


# revision 28
# speedup vs baseline: 1.0093x; 1.0093x over previous
"""MoE layer (E=8 experts, top-2, D=1024, H=4096, N=4096 tokens) on 8 TRN2
NeuronCores.

Strategy: expert-parallel with load balancing. The router (gate matmul +
softmax + top-2) is ~0.1% of the FLOPs and runs on host in float64
(verified to reproduce the reference's f32 top-2 selection exactly). The
host gathers each expert's tokens and ships them to the cores; each core
runs dense single-expert FFN passes over fixed-size token slots:

    outT = w2.T @ gelu(w1.T @ xT + b1)

Only the top-2 experts per token are computed (vs all 8 in the dense
data-parallel formulation) — a ~3.6x FLOP cut. The host applies the
combine weights and the (comb-weighted) b2 bias during the scatter-add
back to token order.

Load balancing: expert loads vary (~944..1091 here), so one-expert-per-
core would pad every core to the max. Instead experts are sorted by load
and paired big-with-small; each pair is split across two cores, so every
core processes slot1 = half of a heavy expert (s1 tokens) + slot2 = half
of a light expert (s2 tokens), with uniform (s1, s2) across cores (SPMD:
one NEFF). Capacity s1+s2 ~= ceil(max_heavy/2) + ceil(max_light/2) beats
max_e count_e.

Layout: activations keep the feature dim on partitions (tokens on the
free axis) so both matmuls consume pre-tiled weights with no on-device
transposes:
    hT[h, t]   = sum_d w1[d, h] * xT[d, t]     (lhsT = w1 tile, rhs = xT)
    outT[d, t] = sum_h w2[h, d] * geluT[h, t]  (lhsT = w2 tile, rhs = geluT)

Matmuls run in bf16 (1 PE row/cycle, FWL weight loads) with fp32 PSUM
accumulation; measured end-to-end error vs the f32 reference is ~3.4e-3
absmax-relative, well under the 2e-2 gate.

Perf notes (measured on TRN2):
 - PE back-to-back MM issue gap = N/2.4GHz + ~2.5ns; LDWEIGHTS fully
   hidden by the PE reorder window. Keep SBUF row pitches and chunk
   starts 16B-aligned or streaming slows ~1.2x.
 - DMA instructions issue serially (~0.65us each) per issuing queue
   (sync/scalar); transfers stripe over 16 HW queues at ~190 GB/s
   aggregate. Emission order is chosen so nothing the PE needs early
   contends with bulk weight traffic.
 - Engines start ~7.2us into the NEFF; a short train of dependency-free
   warmup matmuls keeps the PE clock (HAM) at 2.4 GHz from the start.
"""

import numpy as np
import ml_dtypes

import concourse.bass as bass  # noqa: F401  (bass types used via tile/bacc)
import concourse.mybir as mybir
import concourse.tile as tile
from concourse import bacc, bass_utils

F32 = mybir.dt.float32
BF16 = mybir.dt.bfloat16
AFT = mybir.ActivationFunctionType
NPBF16 = np.dtype(ml_dtypes.bfloat16)

E = 8          # experts
D = 1024       # model dim
H = 4096       # expert hidden dim
P = 128        # partitions
NCORES = 8
NTOK = 4096    # total tokens (B*T = 2*2048)
KD = D // P    # 8 contraction chunks of D
NH = H // P    # 32 h tiles
ND = D // P    # 8 d tiles
NSLOT = 2      # expert slots per core (big-half + small-half)

_NC = {}       # compiled modules keyed by (s1, s2)


def _chunks(s):
    """Split a slot of s tokens into psum-sized chunks (<=512 f32/bank),
    8-aligned starts so SBUF addresses stay 16B-aligned in bf16."""
    if s <= 512:
        return [s]
    a = -(-(s // 2) // 8) * 8
    return [a, s - a]


def _build(s1, s2):
    c = s1 + s2
    slot_off = [0, s1]
    slot_chunks = [_chunks(s1), _chunks(s2)]
    nc = bacc.Bacc("TRN2", target_bir_lowering=False, debug=False,
                   num_devices=NCORES)
    xT = nc.dram_tensor("xT", [P, KD, c], BF16, kind="ExternalInput").ap()
    w1t = nc.dram_tensor("w1t", [NSLOT, NH, P, KD, P], BF16,
                         kind="ExternalInput").ap()
    b1t = nc.dram_tensor("b1t", [P, NSLOT, NH], F32,
                         kind="ExternalInput").ap()
    w2t = nc.dram_tensor("w2t", [NSLOT, ND, P, NH, P], BF16,
                         kind="ExternalInput").ap()
    outT = nc.dram_tensor("outT", [P, ND, c], F32, kind="ExternalOutput").ap()

    with tile.TileContext(nc) as tc:
        with (
            tc.tile_pool(name="const", bufs=1) as cpool,
            tc.tile_pool(name="w1p", bufs=NH) as w1p,
            tc.tile_pool(name="w2p", bufs=2) as w2p,
            tc.tile_pool(name="otp", bufs=4) as otp,
            tc.tile_pool(name="ps", bufs=7, space="PSUM") as ps,
            tc.tile_pool(name="pw", bufs=1, space="PSUM") as pw,
        ):
            # ---- PE warmup: dependency-free matmuls from ~0.5us after
            # engine start keep the HAM activity window busy during the
            # input fill, so the real stream starts at 2.4 GHz.
            wsrc = cpool.tile([P, 256], BF16)
            nc.vector.memset(wsrc[:], 0.0)
            pwt = pw.tile([P, P], F32)
            for _ in range(38):
                nc.tensor.matmul(pwt[:], wsrc[:, 0:P], wsrc[:, P:256],
                                 start=True, stop=True)

            # ---- input DMAs. scalar queue: b1 + slot-1 xT (needed first);
            # sync queue: slot-1 w1 tiles, then slot-2 xT, then slot-2 w1
            # (its ring slots free up as slot-1 mm1 consumes tiles), then
            # (emitted in the loops below) w2 tiles and outputs.
            b1_s = cpool.tile([P, NSLOT, NH], F32)
            nc.scalar.dma_start(b1_s[:], b1t[:])
            xTt = cpool.tile([P, KD, c], BF16)
            for kd in range(KD):
                nc.scalar.dma_start(xTt[:, kd, 0:s1], xT[:, kd, 0:s1])
            w1_all = {}
            for s in range(NSLOT):
                if s == 1:
                    for kd in range(KD):
                        nc.sync.dma_start(xTt[:, kd, s1:c], xT[:, kd, s1:c])
                for h in range(NH):
                    w1_s = w1p.tile([P, KD, P], BF16)
                    nc.sync.dma_start(w1_s[:], w1t[s, h])
                    w1_all[s, h] = w1_s
            geluT = cpool.tile([P, NH, c], BF16)

            # ---- mm1 + gelu: geluT[h, t] = gelu(sum_d w1[d,h] x[d,t] + b1)
            for s in range(NSLOT):
                for h in range(NH):
                    w1_s = w1_all.pop((s, h))
                    t0 = slot_off[s]
                    for cw in slot_chunks[s]:
                        ts = slice(t0, t0 + cw)
                        t0 += cw
                        ph = ps.tile([P, cw], F32, tag="ps")
                        for kd in range(KD):
                            nc.tensor.matmul(ph[:], w1_s[:, kd, :],
                                             xTt[:, kd, ts],
                                             start=(kd == 0),
                                             stop=(kd == KD - 1))
                        nc.scalar.activation(geluT[:, h, ts], ph[:], AFT.Gelu,
                                             bias=b1_s[:, s, h:h + 1])

            # ---- mm2: outT[d, t] = sum_h w2[h,d] geluT[h,t]
            for s in range(NSLOT):
                for d in range(ND):
                    w2_s = w2p.tile([P, NH, P], BF16)
                    for q in range(2):
                        nc.sync.dma_start(
                            w2_s[:, q * NH // 2:(q + 1) * NH // 2, :],
                            w2t[s, d, :, q * NH // 2:(q + 1) * NH // 2, :])
                    t0 = slot_off[s]
                    for cw in slot_chunks[s]:
                        ts = slice(t0, t0 + cw)
                        t0 += cw
                        po = ps.tile([P, cw], F32, tag="ps")
                        for hh in range(NH):
                            nc.tensor.matmul(po[:], w2_s[:, hh, :],
                                             geluT[:, hh, ts],
                                             start=(hh == 0),
                                             stop=(hh == NH - 1))
                        ot = otp.tile([P, cw], F32, tag="ot")
                        nc.vector.tensor_copy(ot[:], po[:])
                        nc.sync.dma_start(outT[:, d, ts], ot[:])

    nc.compile()
    return nc


def _get_nc(s1, s2):
    if (s1, s2) not in _NC:
        _NC[(s1, s2)] = _build(s1, s2)
    return _NC[(s1, s2)]


def _route(xf, gate_w, gate_b):
    """Top-2 routing in float64 (reproduces the reference's f32 decisions)."""
    lg = xf.astype(np.float64) @ gate_w.astype(np.float64) \
        + gate_b.astype(np.float64)
    lg -= lg.max(-1, keepdims=True)
    g = np.exp(lg)
    g /= g.sum(-1, keepdims=True)
    ti = np.argsort(-g, axis=-1, kind="stable")[:, :2]     # [N, 2] desc
    tg = np.take_along_axis(g, ti, axis=1)
    tg = tg / (tg.sum(-1, keepdims=True) + 1e-9)           # combine weights
    return ti, tg


def _wtiles(w1, b1, w2, e):
    w1te = np.ascontiguousarray(
        w1[e].reshape(KD, P, NH, P).transpose(2, 1, 0, 3)).astype(NPBF16)
    b1te = np.ascontiguousarray(b1[e].reshape(NH, P).T)
    w2te = np.ascontiguousarray(
        w2[e].reshape(NH, P, ND, P).transpose(2, 1, 0, 3)).astype(NPBF16)
    return w1te, b1te, w2te


def _prep(x, gate_w, gate_b, w1, b1, w2, b2):
    f = np.float32
    xf = np.asarray(x, f).reshape(NTOK, D)
    gate_w = np.asarray(gate_w, f)
    gate_b = np.asarray(gate_b, f)
    w1 = np.asarray(w1, f)
    b1 = np.asarray(b1, f)
    w2 = np.asarray(w2, f)
    b2 = np.asarray(b2, f)

    ti, tg = _route(xf, gate_w, gate_b)

    sels, wts = [], []
    for e in range(E):
        m = (ti == e)
        sel = np.nonzero(m.any(1))[0]                       # token ids, asc
        wt = tg[sel, m[sel].argmax(1)].astype(f)            # combine weight
        sels.append(sel)
        wts.append(wt)
    counts = np.array([len(s) for s in sels])

    # pair heavy experts with light ones; each pair spans two cores
    order = np.argsort(-counts, kind="stable")
    bigs, smalls = order[:E // 2], order[E // 2:]
    def _slot(maxc):                       # ceil(max/2), rounded up to 8
        return -(-((int(maxc) + 1) // 2) // 8) * 8
    s1 = _slot(counts[bigs].max())
    s2 = _slot(counts[smalls].max())
    c = s1 + s2

    wcache = {}
    in_maps, meta = [], []
    for p in range(E // 2):
        ea, eb = int(bigs[p]), int(smalls[p])
        for e in (ea, eb):
            if e not in wcache:
                wcache[e] = _wtiles(w1, b1, w2, e)
        halves = []
        for e in (ea, eb):
            n = len(sels[e])
            h1 = (n + 1) // 2
            halves.append([(sels[e][:h1], wts[e][:h1]),
                           (sels[e][h1:], wts[e][h1:])])
        for half in range(2):
            (sel_a, wt_a), (sel_b, wt_b) = halves[0][half], halves[1][half]
            xe = np.zeros((c, D), f)
            xe[0:len(sel_a)] = xf[sel_a]
            xe[s1:s1 + len(sel_b)] = xf[sel_b]
            xTe = np.ascontiguousarray(
                xe.T.reshape(KD, P, c).transpose(1, 0, 2)).astype(NPBF16)
            w1te = np.stack([wcache[ea][0], wcache[eb][0]])
            b1te = np.ascontiguousarray(
                np.stack([wcache[ea][1], wcache[eb][1]]).transpose(1, 0, 2))
            w2te = np.stack([wcache[ea][2], wcache[eb][2]])
            in_maps.append({"xT": xTe, "w1t": w1te, "b1t": b1te,
                            "w2t": w2te})
            meta.append(((ea, sel_a, wt_a, 0), (eb, sel_b, wt_b, s1)))
    return in_maps, meta, b2, s1, s2


def _assemble(results, meta, b2):
    out = np.zeros((NTOK, D), np.float32)
    for core, slots in enumerate(meta):
        yT = np.asarray(results[core]["outT"])              # [P, ND, c] f32
        y = yT.transpose(1, 0, 2).reshape(D, -1).T          # [c, D]
        for e, sel, wt, off in slots:
            if len(sel):
                out[sel] += wt[:, None] * (y[off:off + len(sel)] + b2[e])
    return out.reshape(2, NTOK // 2, D)


def run(inputs, trace=False):
    """Run the kernel; returns (output, exec_time_ns or None)."""
    in_maps, meta, b2, s1, s2 = _prep(**inputs)
    nc = _get_nc(s1, s2)
    res = bass_utils.run_bass_kernel_spmd(
        nc, in_maps, core_ids=list(range(NCORES)), trace=trace)
    return _assemble(res.results, meta, b2), res.exec_time_ns


def kernel(**inputs):
    out, _ = run(inputs, trace=False)
    return out


# revision 29
# speedup vs baseline: 1.0104x; 1.0011x over previous
"""MoE layer (E=8 experts, top-2, D=1024, H=4096, N=4096 tokens) on 8 TRN2
NeuronCores.

Strategy: expert-parallel with load balancing. The router (gate matmul +
softmax + top-2) is ~0.1% of the FLOPs and runs on host in float64
(verified to reproduce the reference's f32 top-2 selection exactly). The
host gathers each expert's tokens and ships them to the cores; each core
runs dense single-expert FFN passes over fixed-size token slots:

    outT = w2.T @ gelu(w1.T @ xT + b1)

Only the top-2 experts per token are computed (vs all 8 in the dense
data-parallel formulation) — a ~3.6x FLOP cut. The host applies the
combine weights and the (comb-weighted) b2 bias during the scatter-add
back to token order.

Load balancing: expert loads vary (~944..1091 here), so one-expert-per-
core would pad every core to the max. Instead experts are sorted by load
and paired big-with-small; each pair is split across two cores, so every
core processes slot1 = half of a heavy expert (s1 tokens) + slot2 = half
of a light expert (s2 tokens), with uniform (s1, s2) across cores (SPMD:
one NEFF). Capacity s1+s2 ~= ceil(max_heavy/2) + ceil(max_light/2) beats
max_e count_e.

Layout: activations keep the feature dim on partitions (tokens on the
free axis) so both matmuls consume pre-tiled weights with no on-device
transposes:
    hT[h, t]   = sum_d w1[d, h] * xT[d, t]     (lhsT = w1 tile, rhs = xT)
    outT[d, t] = sum_h w2[h, d] * geluT[h, t]  (lhsT = w2 tile, rhs = geluT)

Matmuls run in bf16 (1 PE row/cycle, FWL weight loads) with fp32 PSUM
accumulation; measured end-to-end error vs the f32 reference is ~3.4e-3
absmax-relative, well under the 2e-2 gate.

Perf notes (measured on TRN2):
 - PE back-to-back MM issue gap = N/2.4GHz + ~2.5ns; LDWEIGHTS fully
   hidden by the PE reorder window. Keep SBUF row pitches and chunk
   starts 16B-aligned or streaming slows ~1.2x.
 - DMA instructions issue serially (~0.65us each) per issuing queue
   (sync/scalar); transfers stripe over 16 HW queues at ~190 GB/s
   aggregate. Emission order is chosen so nothing the PE needs early
   contends with bulk weight traffic.
 - Engines start ~7.2us into the NEFF; a short train of dependency-free
   warmup matmuls keeps the PE clock (HAM) at 2.4 GHz from the start.
"""

import numpy as np
import ml_dtypes

import concourse.bass as bass  # noqa: F401  (bass types used via tile/bacc)
import concourse.mybir as mybir
import concourse.tile as tile
from concourse import bacc, bass_utils

F32 = mybir.dt.float32
BF16 = mybir.dt.bfloat16
AFT = mybir.ActivationFunctionType
NPBF16 = np.dtype(ml_dtypes.bfloat16)

E = 8          # experts
D = 1024       # model dim
H = 4096       # expert hidden dim
P = 128        # partitions
NCORES = 8
NTOK = 4096    # total tokens (B*T = 2*2048)
KD = D // P    # 8 contraction chunks of D
NH = H // P    # 32 h tiles
ND = D // P    # 8 d tiles
NSLOT = 2      # expert slots per core (big-half + small-half)

_NC = {}       # compiled modules keyed by (s1, s2)


def _chunks(s):
    """Split a slot of s tokens into psum-sized chunks (<=512 f32/bank),
    8-aligned starts so SBUF addresses stay 16B-aligned in bf16."""
    if s <= 512:
        return [s]
    a = -(-(s // 2) // 8) * 8
    return [a, s - a]


def _build(s1, s2):
    c = s1 + s2
    slot_off = [0, s1]
    slot_chunks = [_chunks(s1), _chunks(s2)]
    nc = bacc.Bacc("TRN2", target_bir_lowering=False, debug=False,
                   num_devices=NCORES)
    xT = nc.dram_tensor("xT", [P, KD, c], BF16, kind="ExternalInput").ap()
    w1t = nc.dram_tensor("w1t", [NSLOT, NH, P, KD, P], BF16,
                         kind="ExternalInput").ap()
    b1t = nc.dram_tensor("b1t", [P, NSLOT, NH], F32,
                         kind="ExternalInput").ap()
    w2t = nc.dram_tensor("w2t", [NSLOT, ND, P, NH, P], BF16,
                         kind="ExternalInput").ap()
    outT = nc.dram_tensor("outT", [P, ND, c], F32, kind="ExternalOutput").ap()

    with tile.TileContext(nc) as tc:
        with (
            tc.tile_pool(name="const", bufs=1) as cpool,
            tc.tile_pool(name="w1p", bufs=NH) as w1p,
            tc.tile_pool(name="w2p", bufs=2) as w2p,
            tc.tile_pool(name="otp", bufs=4) as otp,
            tc.tile_pool(name="ps", bufs=7, space="PSUM") as ps,
            tc.tile_pool(name="pw", bufs=1, space="PSUM") as pw,
        ):
            # ---- PE warmup: dependency-free matmuls from ~0.5us after
            # engine start keep the HAM activity window busy during the
            # input fill, so the real stream starts at 2.4 GHz.
            wsrc = cpool.tile([P, 256], BF16)
            nc.vector.memset(wsrc[:], 0.0)
            pwt = pw.tile([P, P], F32)
            for _ in range(38):
                nc.tensor.matmul(pwt[:], wsrc[:, 0:P], wsrc[:, P:256],
                                 start=True, stop=True)

            # ---- input DMAs. Slot-1 xT is what the first matmul groups
            # consume kd-by-kd, so its 8 issues split across BOTH queues
            # (scalar + sync) ahead of the w1 chain — serial issue on one
            # queue would starve the early kd stream and let the HAM
            # re-throttle. Then sync: slot-1 w1 tiles, slot-2 xT, slot-2 w1
            # (its ring slots free as slot-1 mm1 consumes), then (emitted in
            # the loops below) w2 tiles and outputs.
            b1_s = cpool.tile([P, NSLOT, NH], F32)
            nc.scalar.dma_start(b1_s[:], b1t[:])
            xTt = cpool.tile([P, KD, c], BF16)
            for kd in range(KD // 2):
                nc.scalar.dma_start(xTt[:, kd, 0:s1], xT[:, kd, 0:s1])
            for kd in range(KD // 2, KD):
                nc.sync.dma_start(xTt[:, kd, 0:s1], xT[:, kd, 0:s1])
            w1_all = {}
            for s in range(NSLOT):
                if s == 1:
                    for kd in range(KD):
                        nc.sync.dma_start(xTt[:, kd, s1:c], xT[:, kd, s1:c])
                for h in range(NH):
                    w1_s = w1p.tile([P, KD, P], BF16)
                    nc.sync.dma_start(w1_s[:], w1t[s, h])
                    w1_all[s, h] = w1_s
            geluT = cpool.tile([P, NH, c], BF16)

            # ---- mm1 + gelu: geluT[h, t] = gelu(sum_d w1[d,h] x[d,t] + b1)
            for s in range(NSLOT):
                for h in range(NH):
                    w1_s = w1_all.pop((s, h))
                    t0 = slot_off[s]
                    for cw in slot_chunks[s]:
                        ts = slice(t0, t0 + cw)
                        t0 += cw
                        ph = ps.tile([P, cw], F32, tag="ps")
                        for kd in range(KD):
                            nc.tensor.matmul(ph[:], w1_s[:, kd, :],
                                             xTt[:, kd, ts],
                                             start=(kd == 0),
                                             stop=(kd == KD - 1))
                        nc.scalar.activation(geluT[:, h, ts], ph[:], AFT.Gelu,
                                             bias=b1_s[:, s, h:h + 1])

            # ---- mm2: outT[d, t] = sum_h w2[h,d] geluT[h,t]
            for s in range(NSLOT):
                for d in range(ND):
                    w2_s = w2p.tile([P, NH, P], BF16)
                    for q in range(2):
                        nc.sync.dma_start(
                            w2_s[:, q * NH // 2:(q + 1) * NH // 2, :],
                            w2t[s, d, :, q * NH // 2:(q + 1) * NH // 2, :])
                    t0 = slot_off[s]
                    for cw in slot_chunks[s]:
                        ts = slice(t0, t0 + cw)
                        t0 += cw
                        po = ps.tile([P, cw], F32, tag="ps")
                        for hh in range(NH):
                            nc.tensor.matmul(po[:], w2_s[:, hh, :],
                                             geluT[:, hh, ts],
                                             start=(hh == 0),
                                             stop=(hh == NH - 1))
                        ot = otp.tile([P, cw], F32, tag="ot")
                        nc.vector.tensor_copy(ot[:], po[:])
                        nc.sync.dma_start(outT[:, d, ts], ot[:])

    nc.compile()
    return nc


def _get_nc(s1, s2):
    if (s1, s2) not in _NC:
        _NC[(s1, s2)] = _build(s1, s2)
    return _NC[(s1, s2)]


def _route(xf, gate_w, gate_b):
    """Top-2 routing in float64 (reproduces the reference's f32 decisions)."""
    lg = xf.astype(np.float64) @ gate_w.astype(np.float64) \
        + gate_b.astype(np.float64)
    lg -= lg.max(-1, keepdims=True)
    g = np.exp(lg)
    g /= g.sum(-1, keepdims=True)
    ti = np.argsort(-g, axis=-1, kind="stable")[:, :2]     # [N, 2] desc
    tg = np.take_along_axis(g, ti, axis=1)
    tg = tg / (tg.sum(-1, keepdims=True) + 1e-9)           # combine weights
    return ti, tg


def _wtiles(w1, b1, w2, e):
    w1te = np.ascontiguousarray(
        w1[e].reshape(KD, P, NH, P).transpose(2, 1, 0, 3)).astype(NPBF16)
    b1te = np.ascontiguousarray(b1[e].reshape(NH, P).T)
    w2te = np.ascontiguousarray(
        w2[e].reshape(NH, P, ND, P).transpose(2, 1, 0, 3)).astype(NPBF16)
    return w1te, b1te, w2te


def _prep(x, gate_w, gate_b, w1, b1, w2, b2):
    f = np.float32
    xf = np.asarray(x, f).reshape(NTOK, D)
    gate_w = np.asarray(gate_w, f)
    gate_b = np.asarray(gate_b, f)
    w1 = np.asarray(w1, f)
    b1 = np.asarray(b1, f)
    w2 = np.asarray(w2, f)
    b2 = np.asarray(b2, f)

    ti, tg = _route(xf, gate_w, gate_b)

    sels, wts = [], []
    for e in range(E):
        m = (ti == e)
        sel = np.nonzero(m.any(1))[0]                       # token ids, asc
        wt = tg[sel, m[sel].argmax(1)].astype(f)            # combine weight
        sels.append(sel)
        wts.append(wt)
    counts = np.array([len(s) for s in sels])

    # pair heavy experts with light ones; each pair spans two cores
    order = np.argsort(-counts, kind="stable")
    bigs, smalls = order[:E // 2], order[E // 2:]
    def _slot(maxc):                       # ceil(max/2), rounded up to 8
        return -(-((int(maxc) + 1) // 2) // 8) * 8
    s1 = _slot(counts[bigs].max())
    s2 = _slot(counts[smalls].max())
    c = s1 + s2

    wcache = {}
    in_maps, meta = [], []
    for p in range(E // 2):
        ea, eb = int(bigs[p]), int(smalls[p])
        for e in (ea, eb):
            if e not in wcache:
                wcache[e] = _wtiles(w1, b1, w2, e)
        halves = []
        for e in (ea, eb):
            n = len(sels[e])
            h1 = (n + 1) // 2
            halves.append([(sels[e][:h1], wts[e][:h1]),
                           (sels[e][h1:], wts[e][h1:])])
        for half in range(2):
            (sel_a, wt_a), (sel_b, wt_b) = halves[0][half], halves[1][half]
            xe = np.zeros((c, D), f)
            xe[0:len(sel_a)] = xf[sel_a]
            xe[s1:s1 + len(sel_b)] = xf[sel_b]
            xTe = np.ascontiguousarray(
                xe.T.reshape(KD, P, c).transpose(1, 0, 2)).astype(NPBF16)
            w1te = np.stack([wcache[ea][0], wcache[eb][0]])
            b1te = np.ascontiguousarray(
                np.stack([wcache[ea][1], wcache[eb][1]]).transpose(1, 0, 2))
            w2te = np.stack([wcache[ea][2], wcache[eb][2]])
            in_maps.append({"xT": xTe, "w1t": w1te, "b1t": b1te,
                            "w2t": w2te})
            meta.append(((ea, sel_a, wt_a, 0), (eb, sel_b, wt_b, s1)))
    return in_maps, meta, b2, s1, s2


def _assemble(results, meta, b2):
    out = np.zeros((NTOK, D), np.float32)
    for core, slots in enumerate(meta):
        yT = np.asarray(results[core]["outT"])              # [P, ND, c] f32
        y = yT.transpose(1, 0, 2).reshape(D, -1).T          # [c, D]
        for e, sel, wt, off in slots:
            if len(sel):
                out[sel] += wt[:, None] * (y[off:off + len(sel)] + b2[e])
    return out.reshape(2, NTOK // 2, D)


def run(inputs, trace=False):
    """Run the kernel; returns (output, exec_time_ns or None)."""
    in_maps, meta, b2, s1, s2 = _prep(**inputs)
    nc = _get_nc(s1, s2)
    res = bass_utils.run_bass_kernel_spmd(
        nc, in_maps, core_ids=list(range(NCORES)), trace=trace)
    return _assemble(res.results, meta, b2), res.exec_time_ns


def kernel(**inputs):
    out, _ = run(inputs, trace=False)
    return out


# revision 30
# speedup vs baseline: 1.0204x; 1.0099x over previous
"""MoE layer (E=8 experts, top-2, D=1024, H=4096, N=4096 tokens) on 8 TRN2
NeuronCores.

Strategy: expert-parallel with load balancing. The router (gate matmul +
softmax + top-2) is ~0.1% of the FLOPs and runs on host in float64
(verified to reproduce the reference's f32 top-2 selection exactly). The
host gathers each expert's tokens and ships them to the cores; each core
runs dense single-expert FFN passes over fixed-size token slots:

    outT = w2.T @ gelu(w1.T @ xT + b1)

Only the top-2 experts per token are computed (vs all 8 in the dense
data-parallel formulation) — a ~3.6x FLOP cut. The host applies the
combine weights and the (comb-weighted) b2 bias during the scatter-add
back to token order.

Load balancing: expert loads vary (~944..1091 here), so one-expert-per-
core would pad every core to the max. Instead experts are sorted by load
and paired big-with-small; each pair is split across two cores, so every
core processes slot1 = half of a heavy expert (s1 tokens) + slot2 = half
of a light expert (s2 tokens), with uniform (s1, s2) across cores (SPMD:
one NEFF). Capacity s1+s2 ~= ceil(max_heavy/2) + ceil(max_light/2) beats
max_e count_e.

Layout: activations keep the feature dim on partitions (tokens on the
free axis) so both matmuls consume pre-tiled weights with no on-device
transposes:
    hT[h, t]   = sum_d w1[d, h] * xT[d, t]     (lhsT = w1 tile, rhs = xT)
    outT[d, t] = sum_h w2[h, d] * geluT[h, t]  (lhsT = w2 tile, rhs = geluT)

Matmuls run in bf16 (1 PE row/cycle, FWL weight loads) with fp32 PSUM
accumulation; measured end-to-end error vs the f32 reference is ~3.4e-3
absmax-relative, well under the 2e-2 gate.

Perf notes (measured on TRN2):
 - PE back-to-back MM issue gap = N/2.4GHz + ~2.5ns; LDWEIGHTS fully
   hidden by the PE reorder window. Keep SBUF row pitches and chunk
   starts 16B-aligned or streaming slows ~1.2x.
 - DMA instructions issue serially (~0.65us each) per issuing queue
   (sync/scalar); transfers stripe over 16 HW queues at ~190 GB/s
   aggregate. Emission order is chosen so nothing the PE needs early
   contends with bulk weight traffic.
 - Engines start ~7.2us into the NEFF; a short train of dependency-free
   warmup matmuls keeps the PE clock (HAM) at 2.4 GHz from the start.
"""

import numpy as np
import ml_dtypes

import concourse.bass as bass  # noqa: F401  (bass types used via tile/bacc)
import concourse.mybir as mybir
import concourse.tile as tile
from concourse import bacc, bass_utils

F32 = mybir.dt.float32
BF16 = mybir.dt.bfloat16
AFT = mybir.ActivationFunctionType
NPBF16 = np.dtype(ml_dtypes.bfloat16)

E = 8          # experts
D = 1024       # model dim
H = 4096       # expert hidden dim
P = 128        # partitions
NCORES = 8
NTOK = 4096    # total tokens (B*T = 2*2048)
KD = D // P    # 8 contraction chunks of D
NH = H // P    # 32 h tiles
ND = D // P    # 8 d tiles
NSLOT = 2      # expert slots per core (big-half + small-half)

_NC = {}       # compiled modules keyed by (s1, s2)


def _chunks(s):
    """Split a slot of s tokens into psum-sized chunks (<=512 f32/bank),
    8-aligned starts so SBUF addresses stay 16B-aligned in bf16."""
    if s <= 512:
        return [s]
    a = -(-(s // 2) // 8) * 8
    return [a, s - a]


def _build(s1, s2):
    c = s1 + s2
    slot_off = [0, s1]
    slot_chunks = [_chunks(s1), _chunks(s2)]
    nc = bacc.Bacc("TRN2", target_bir_lowering=False, debug=False,
                   num_devices=NCORES)
    xT = nc.dram_tensor("xT", [P, KD, c], BF16, kind="ExternalInput").ap()
    w1t = nc.dram_tensor("w1t", [NSLOT, NH, P, KD, P], BF16,
                         kind="ExternalInput").ap()
    b1t = nc.dram_tensor("b1t", [P, NSLOT, NH], F32,
                         kind="ExternalInput").ap()
    w2t = nc.dram_tensor("w2t", [NSLOT, ND, P, NH, P], BF16,
                         kind="ExternalInput").ap()
    outT = nc.dram_tensor("outT", [P, ND, c], F32, kind="ExternalOutput").ap()

    with tile.TileContext(nc) as tc:
        with (
            tc.tile_pool(name="const", bufs=1) as cpool,
            tc.tile_pool(name="w1p", bufs=NH) as w1p,
            tc.tile_pool(name="w2p", bufs=2) as w2p,
            tc.tile_pool(name="otp", bufs=4) as otp,
            tc.tile_pool(name="ps", bufs=7, space="PSUM") as ps,
            tc.tile_pool(name="pw", bufs=1, space="PSUM") as pw,
        ):
            # ---- PE warmup: dependency-free matmuls from ~0.5us after
            # engine start keep the HAM activity window busy during the
            # input fill, so the real stream starts at 2.4 GHz.
            wsrc = cpool.tile([P, 256], BF16)
            nc.vector.memset(wsrc[:], 0.0)
            pwt = pw.tile([P, P], F32)
            for _ in range(38):
                nc.tensor.matmul(pwt[:], wsrc[:, 0:P], wsrc[:, P:256],
                                 start=True, stop=True)

            # ---- input DMAs. Slot-1 xT is what the first matmul groups
            # consume kd-by-kd, so its 8 issues split across BOTH queues
            # (scalar + sync) ahead of the w1 chain — serial issue on one
            # queue would starve the early kd stream and let the HAM
            # re-throttle. Then sync: slot-1 w1 tiles, slot-2 xT, slot-2 w1
            # (its ring slots free as slot-1 mm1 consumes), then (emitted in
            # the loops below) w2 tiles and outputs.
            b1_s = cpool.tile([P, NSLOT, NH], F32)
            nc.scalar.dma_start(b1_s[:], b1t[:])
            xTt = cpool.tile([P, KD, c], BF16)
            w1_all = {}
            for h in range(2):                    # first h-groups' weights
                w1_s = w1p.tile([P, KD, P], BF16)
                nc.sync.dma_start(w1_s[:], w1t[0, h])
                w1_all[0, h] = w1_s
            for kd in range(KD // 2):
                nc.scalar.dma_start(xTt[:, kd, 0:s1], xT[:, kd, 0:s1])
            for kd in range(KD // 2, KD):
                nc.sync.dma_start(xTt[:, kd, 0:s1], xT[:, kd, 0:s1])
            for s in range(NSLOT):
                if s == 1:
                    for kd in range(KD):
                        nc.sync.dma_start(xTt[:, kd, s1:c], xT[:, kd, s1:c])
                for h in range(2 if s == 0 else 0, NH):
                    w1_s = w1p.tile([P, KD, P], BF16)
                    nc.sync.dma_start(w1_s[:], w1t[s, h])
                    w1_all[s, h] = w1_s
            geluT = cpool.tile([P, NH, c], BF16)

            # ---- mm1 + gelu: geluT[h, t] = gelu(sum_d w1[d,h] x[d,t] + b1)
            for s in range(NSLOT):
                for h in range(NH):
                    w1_s = w1_all.pop((s, h))
                    t0 = slot_off[s]
                    for cw in slot_chunks[s]:
                        ts = slice(t0, t0 + cw)
                        t0 += cw
                        ph = ps.tile([P, cw], F32, tag="ps")
                        for kd in range(KD):
                            nc.tensor.matmul(ph[:], w1_s[:, kd, :],
                                             xTt[:, kd, ts],
                                             start=(kd == 0),
                                             stop=(kd == KD - 1))
                        nc.scalar.activation(geluT[:, h, ts], ph[:], AFT.Gelu,
                                             bias=b1_s[:, s, h:h + 1])

            # ---- mm2: outT[d, t] = sum_h w2[h,d] geluT[h,t]
            for s in range(NSLOT):
                for d in range(ND):
                    w2_s = w2p.tile([P, NH, P], BF16)
                    for q in range(2):
                        nc.sync.dma_start(
                            w2_s[:, q * NH // 2:(q + 1) * NH // 2, :],
                            w2t[s, d, :, q * NH // 2:(q + 1) * NH // 2, :])
                    t0 = slot_off[s]
                    for cw in slot_chunks[s]:
                        ts = slice(t0, t0 + cw)
                        t0 += cw
                        po = ps.tile([P, cw], F32, tag="ps")
                        for hh in range(NH):
                            nc.tensor.matmul(po[:], w2_s[:, hh, :],
                                             geluT[:, hh, ts],
                                             start=(hh == 0),
                                             stop=(hh == NH - 1))
                        ot = otp.tile([P, cw], F32, tag="ot")
                        nc.vector.tensor_copy(ot[:], po[:])
                        nc.sync.dma_start(outT[:, d, ts], ot[:])

    nc.compile()
    return nc


def _get_nc(s1, s2):
    if (s1, s2) not in _NC:
        _NC[(s1, s2)] = _build(s1, s2)
    return _NC[(s1, s2)]


def _route(xf, gate_w, gate_b):
    """Top-2 routing in float64 (reproduces the reference's f32 decisions)."""
    lg = xf.astype(np.float64) @ gate_w.astype(np.float64) \
        + gate_b.astype(np.float64)
    lg -= lg.max(-1, keepdims=True)
    g = np.exp(lg)
    g /= g.sum(-1, keepdims=True)
    ti = np.argsort(-g, axis=-1, kind="stable")[:, :2]     # [N, 2] desc
    tg = np.take_along_axis(g, ti, axis=1)
    tg = tg / (tg.sum(-1, keepdims=True) + 1e-9)           # combine weights
    return ti, tg


def _wtiles(w1, b1, w2, e):
    w1te = np.ascontiguousarray(
        w1[e].reshape(KD, P, NH, P).transpose(2, 1, 0, 3)).astype(NPBF16)
    b1te = np.ascontiguousarray(b1[e].reshape(NH, P).T)
    w2te = np.ascontiguousarray(
        w2[e].reshape(NH, P, ND, P).transpose(2, 1, 0, 3)).astype(NPBF16)
    return w1te, b1te, w2te


def _prep(x, gate_w, gate_b, w1, b1, w2, b2):
    f = np.float32
    xf = np.asarray(x, f).reshape(NTOK, D)
    gate_w = np.asarray(gate_w, f)
    gate_b = np.asarray(gate_b, f)
    w1 = np.asarray(w1, f)
    b1 = np.asarray(b1, f)
    w2 = np.asarray(w2, f)
    b2 = np.asarray(b2, f)

    ti, tg = _route(xf, gate_w, gate_b)

    sels, wts = [], []
    for e in range(E):
        m = (ti == e)
        sel = np.nonzero(m.any(1))[0]                       # token ids, asc
        wt = tg[sel, m[sel].argmax(1)].astype(f)            # combine weight
        sels.append(sel)
        wts.append(wt)
    counts = np.array([len(s) for s in sels])

    # pair heavy experts with light ones; each pair spans two cores
    order = np.argsort(-counts, kind="stable")
    bigs, smalls = order[:E // 2], order[E // 2:]
    def _slot(maxc):                       # ceil(max/2), rounded up to 8
        return -(-((int(maxc) + 1) // 2) // 8) * 8
    s1 = _slot(counts[bigs].max())
    s2 = _slot(counts[smalls].max())
    c = s1 + s2

    wcache = {}
    in_maps, meta = [], []
    for p in range(E // 2):
        ea, eb = int(bigs[p]), int(smalls[p])
        for e in (ea, eb):
            if e not in wcache:
                wcache[e] = _wtiles(w1, b1, w2, e)
        halves = []
        for e in (ea, eb):
            n = len(sels[e])
            h1 = (n + 1) // 2
            halves.append([(sels[e][:h1], wts[e][:h1]),
                           (sels[e][h1:], wts[e][h1:])])
        for half in range(2):
            (sel_a, wt_a), (sel_b, wt_b) = halves[0][half], halves[1][half]
            xe = np.zeros((c, D), f)
            xe[0:len(sel_a)] = xf[sel_a]
            xe[s1:s1 + len(sel_b)] = xf[sel_b]
            xTe = np.ascontiguousarray(
                xe.T.reshape(KD, P, c).transpose(1, 0, 2)).astype(NPBF16)
            w1te = np.stack([wcache[ea][0], wcache[eb][0]])
            b1te = np.ascontiguousarray(
                np.stack([wcache[ea][1], wcache[eb][1]]).transpose(1, 0, 2))
            w2te = np.stack([wcache[ea][2], wcache[eb][2]])
            in_maps.append({"xT": xTe, "w1t": w1te, "b1t": b1te,
                            "w2t": w2te})
            meta.append(((ea, sel_a, wt_a, 0), (eb, sel_b, wt_b, s1)))
    return in_maps, meta, b2, s1, s2


def _assemble(results, meta, b2):
    out = np.zeros((NTOK, D), np.float32)
    for core, slots in enumerate(meta):
        yT = np.asarray(results[core]["outT"])              # [P, ND, c] f32
        y = yT.transpose(1, 0, 2).reshape(D, -1).T          # [c, D]
        for e, sel, wt, off in slots:
            if len(sel):
                out[sel] += wt[:, None] * (y[off:off + len(sel)] + b2[e])
    return out.reshape(2, NTOK // 2, D)


def run(inputs, trace=False):
    """Run the kernel; returns (output, exec_time_ns or None)."""
    in_maps, meta, b2, s1, s2 = _prep(**inputs)
    nc = _get_nc(s1, s2)
    res = bass_utils.run_bass_kernel_spmd(
        nc, in_maps, core_ids=list(range(NCORES)), trace=trace)
    return _assemble(res.results, meta, b2), res.exec_time_ns


def kernel(**inputs):
    out, _ = run(inputs, trace=False)
    return out
